# revision 20
# baseline (speedup 1.0000x reference)
import numpy as np

N = 30000
E = 480000
D = 256
NG = 256
OUT = 10
NCORES = 8
NPC = N // NCORES          # 3750 nodes per core
NPAD = 3840                # 30 groups of 128
GROUPS = NPAD // 128
WCH = 8                    # gather window (chunks per dma_gather)
NSLICE = 8
SL = NPAD // NSLICE        # 480

TRACE = False
LAST_EXEC_NS = []
LAST_WALL_NS = []
_PROG_CACHE = {}


def _cdiv(a, b):
    return (a + b - 1) // b


def _bn(h, g, b):
    import jax
    m = h.mean(axis=0)
    v = h.var(axis=0)
    return g * (h - m) * jax.lax.rsqrt(v + 1e-5) + b


def _edge_prep(src, dst, dis):
    order = np.argsort(dst, kind="stable")
    ssrc = src[order].astype(np.int64)
    sdst = dst[order].astype(np.int64)
    ents = [[None] * GROUPS for _ in range(NCORES)]
    cnts = np.zeros((NCORES, GROUPS), np.int64)
    for c in range(NCORES):
        for g in range(GROUPS):
            lo = c * NPC + g * 128
            hi = min(lo + 128, (c + 1) * NPC)
            if lo >= hi:
                ents[c][g] = (np.zeros(0, np.int64), np.zeros(0, np.int64), np.zeros(0, np.float32))
                continue
            e_lo = np.searchsorted(sdst, lo, "left")
            e_hi = np.searchsorted(sdst, hi, "left")
            es = ssrc[e_lo:e_hi]
            ed = sdst[e_lo:e_hi]
            selfn = np.arange(lo, hi, dtype=np.int64)
            srcs = np.concatenate([es, selfn])
            cols = np.concatenate([ed - lo, selfn - lo])
            dv = np.concatenate([dis[es] * dis[ed], dis[selfn] * dis[selfn]]).astype(np.float32)
            ents[c][g] = (srcs, cols, dv)
            cnts[c, g] = len(srcs)
    chunks = np.maximum(_cdiv(cnts, 128).max(axis=0), 1).astype(np.int64)
    C = _cdiv(int(chunks.sum()), WCH) * WCH  # pad to full gather windows
    idxs_l, colv_l, disv_l = [], [], []
    for c in range(NCORES):
        gsrc = np.zeros(C * 128, np.int64)
        gcol = np.full(C * 128, -1.0, np.float32)
        gdv = np.zeros(C * 128, np.float32)
        off = 0
        for g in range(GROUPS):
            srcs, cols, dv = ents[c][g]
            n = len(srcs)
            gsrc[off:off + n] = srcs
            gcol[off:off + n] = cols.astype(np.float32)
            gdv[off:off + n] = dv
            off += int(chunks[g]) * 128
        idx16 = gsrc.astype(np.int16).reshape(C, 8, 16).transpose(2, 0, 1).reshape(16, C * 8)
        idxs_l.append(np.ascontiguousarray(np.tile(idx16, (8, 1))))
        colv_l.append(np.ascontiguousarray(gcol.reshape(C, 128).T))
        disv_l.append(np.ascontiguousarray(gdv.reshape(C, 128).T))
    return chunks, idxs_l, colv_l, disv_l


def _build_gnn(chunks):
    import concourse.bass as bass
    from concourse import mybir, tile
    from concourse.library_config import mlp as _mlp_lib

    C = _cdiv(int(sum(chunks)), WCH) * WCH
    f32 = mybir.dt.float32
    f32r = mybir.dt.float32r
    AF = mybir.ActivationFunctionType
    ALU = mybir.AluOpType

    nc = bass.Bass("TRN2", target_bir_lowering=False, debug=False, num_devices=NCORES)
    x2full_d = nc.dram_tensor("x2full", [N, D], f32, kind="ExternalInput").ap()
    x2T_d = nc.dram_tensor("x2T", [128, 2, NPAD], f32, kind="ExternalInput").ap()
    invm_d = nc.dram_tensor("invm", [128, NPAD], f32, kind="ExternalInput").ap()
    idxs_d = nc.dram_tensor("idxs", [128, C * 8], mybir.dt.int16, kind="ExternalInput").ap()
    colv_d = nc.dram_tensor("colv", [128, C], f32, kind="ExternalInput").ap()
    disv_d = nc.dram_tensor("disv", [128, C], f32, kind="ExternalInput").ap()
    wnames = ("Wg", "Ws", "Wsn", "Wc", "Rp", "W1", "W2")
    w_d = {n: nc.dram_tensor(n, [128, 2, 2, 128], f32, kind="ExternalInput").ap() for n in wnames}
    b1_d = nc.dram_tensor("b1", [128, 2], f32, kind="ExternalInput").ap()
    ball_d = nc.dram_tensor("ball", [128, 2], f32, kind="ExternalInput").ap()
    iotaf_d = nc.dram_tensor("iotaf", [128, 128], f32, kind="ExternalInput").ap()
    hT_d = nc.dram_tensor("hT", [128, 2, NPAD], f32, kind="ExternalOutput").ap()

    with tile.TileContext(nc) as tc:
        with tc.tile_pool(name="persist", bufs=1) as pp:
            x2T_sb = pp.tile([128, 2, NPAD], f32, tag="x2T", name="x2T_sb")
            PT = pp.tile([128, 2, NPAD], f32, tag="PT", name="PT")
            QT = pp.tile([128, 2, NPAD], f32, tag="QT", name="QT")
            invm_sb = pp.tile([128, NPAD], f32, tag="invm", name="invm_sb")
            idxs_sb = pp.tile([128, C * 8], mybir.dt.int16, tag="idxs", name="idxs_sb")
            colv_sb = pp.tile([128, C], f32, tag="colv", name="colv_sb")
            disv_sb = pp.tile([128, C], f32, tag="disv", name="disv_sb")
            w_sb = {n: pp.tile([128, 2, 2, 128], f32, tag=n, name=n + "_sb") for n in wnames}
            b1_sb = pp.tile([128, 2], f32, tag="b1", name="b1_sb")
            ball_sb = pp.tile([128, 2], f32, tag="ball", name="ball_sb")
            iota_f = pp.tile([128, 128], f32, tag="iota_f", name="iota_f")

            nc.sync.dma_start(out=x2T_sb[:, :, :].bitcast(f32r), in_=x2T_d[:, :, :].bitcast(f32r))
            nc.sync.dma_start(out=invm_sb[:, :], in_=invm_d[:, :])
            nc.sync.dma_start(out=idxs_sb[:, :], in_=idxs_d[:, :])
            nc.sync.dma_start(out=colv_sb[:, :], in_=colv_d[:, :])
            nc.sync.dma_start(out=disv_sb[:, :], in_=disv_d[:, :])
            for n in wnames:
                nc.sync.dma_start(out=w_sb[n][:, :, :, :].bitcast(f32r),
                                  in_=w_d[n][:, :, :, :].bitcast(f32r))
            nc.sync.dma_start(out=b1_sb[:, :], in_=b1_d[:, :])
            nc.sync.dma_start(out=ball_sb[:, :], in_=ball_d[:, :])
            nc.sync.dma_start(out=iota_f[:, :], in_=iotaf_d[:, :])
            nc.gpsimd.load_library(_mlp_lib)
            nidx_reg = nc.gpsimd.to_reg(WCH * 128)

            # ---- aggregation: P' (sum + self) and Q' (sym-norm sum + self) via one-hot matmul
            with tc.tile_pool(name="gat", bufs=2) as gat, \
                 tc.tile_pool(name="ohp", bufs=4) as ohp, \
                 tc.tile_pool(name="aps", bufs=4, space="PSUM") as aps:
                gt = None
                ci = 0
                for g in range(GROUPS):
                    nch = int(chunks[g])
                    ps0 = aps.tile([128, 256], f32, name="ps0")
                    ps1 = aps.tile([128, 256], f32, name="ps1")
                    for j in range(nch):
                        if ci % WCH == 0:
                            gt = gat.tile([128, WCH, 256], f32, name="gt")
                            nc.gpsimd.dma_gather(gt[:, :, :].bitcast(f32r),
                                                 x2full_d[:, :].bitcast(f32r),
                                                 idxs_sb[:, ci * 8:(ci + WCH) * 8],
                                                 WCH * 128, nidx_reg, 256)
                        slot = ci % WCH
                        oh = ohp.tile([128, 256], f32, name="oh")
                        nc.vector.tensor_tensor(out=oh[:, 0:128].bitcast(f32r),
                                                in0=colv_sb[:, ci:ci + 1].to_broadcast([128, 128])[:],
                                                in1=iota_f[:, :], op=ALU.is_equal)
                        nc.scalar.activation(oh[:, 128:256].bitcast(f32r), oh[:, 0:128], AF.Copy,
                                             bias=0.0, scale=disv_sb[:, ci:ci + 1])
                        nc.tensor.matmul(out=ps0[:, :],
                                         lhsT=gt[:, slot, 0:128].bitcast(f32r),
                                         rhs=oh[:, :].bitcast(f32r),
                                         start=(j == 0), stop=(j == nch - 1),
                                         skip_group_check=True)
                        nc.tensor.matmul(out=ps1[:, :],
                                         lhsT=gt[:, slot, 128:256].bitcast(f32r),
                                         rhs=oh[:, :].bitcast(f32r),
                                         start=(j == 0), stop=(j == nch - 1),
                                         skip_group_check=True)
                        ci += 1
                    gsl = slice(g * 128, (g + 1) * 128)
                    nc.vector.tensor_copy(out=PT[:, 0, gsl].bitcast(f32r), in_=ps0[:, 0:128])
                    nc.scalar.copy(QT[:, 0, gsl].bitcast(f32r), ps0[:, 128:256])
                    nc.vector.tensor_copy(out=PT[:, 1, gsl].bitcast(f32r), in_=ps1[:, 0:128])
                    nc.scalar.copy(QT[:, 1, gsl].bitcast(f32r), ps1[:, 128:256])

            # ---- dense phase: h = Q'Wg + P'Wc + x2 Rp + relu(P'W1+b1)W2 + invm*((P'-x2)Ws) + ball
            with tc.tile_pool(name="tsbp", bufs=2) as tsbp, \
                 tc.tile_pool(name="evac", bufs=4) as evacp, \
                 tc.tile_pool(name="tps", bufs=2, space="PSUM") as tpsp, \
                 tc.tile_pool(name="yps", bufs=2, space="PSUM") as ypsp, \
                 tc.tile_pool(name="hps", bufs=2, space="PSUM") as hpsp:
                for s in range(NSLICE):
                    nsl = slice(s * SL, (s + 1) * SL)
                    tsb = tsbp.tile([128, 2, SL], f32, name="tsb")
                    for m in range(2):
                        tps = tpsp.tile([128, SL], f32, name="tps")
                        for k in range(2):
                            nc.tensor.matmul(out=tps[:, :],
                                             lhsT=w_sb["W1"][:, k, m, :].bitcast(f32r),
                                             rhs=PT[:, k, nsl].bitcast(f32r),
                                             start=(k == 0), stop=(k == 1))
                        nc.scalar.activation(tsb[:, m, :].bitcast(f32r), tps[:, :], AF.Relu,
                                             bias=b1_sb[:, m:m + 1], scale=1.0)
                    for m in range(2):
                        yps = ypsp.tile([128, SL], f32, name="yps")
                        q = 0
                        for k in range(2):
                            for wn, rhs_ap in (("Ws", PT[:, k, nsl]), ("Wsn", x2T_sb[:, k, nsl])):
                                nc.tensor.matmul(out=yps[:, :],
                                                 lhsT=w_sb[wn][:, k, m, :].bitcast(f32r),
                                                 rhs=rhs_ap.bitcast(f32r),
                                                 start=(q == 0), stop=(q == 3))
                                q += 1
                        hps = hpsp.tile([128, SL], f32, name="hps")
                        q = 0
                        for k in range(2):
                            for wn, rhs_ap in (("Wg", QT[:, k, nsl]), ("Wc", PT[:, k, nsl]),
                                               ("Rp", x2T_sb[:, k, nsl]), ("W2", tsb[:, k, :])):
                                nc.tensor.matmul(out=hps[:, :],
                                                 lhsT=w_sb[wn][:, k, m, :].bitcast(f32r),
                                                 rhs=rhs_ap.bitcast(f32r),
                                                 start=(q == 0), stop=(q == 7))
                                q += 1
                        ysb = evacp.tile([128, SL], f32, name="ysb")
                        nc.vector.tensor_tensor(out=ysb[:, :], in0=yps[:, :],
                                                in1=invm_sb[:, nsl], op=ALU.mult)
                        h1sb = evacp.tile([128, SL], f32, name="h1sb")
                        nc.scalar.activation(h1sb[:, :], hps[:, :], AF.Identity,
                                             bias=ball_sb[:, m:m + 1], scale=1.0)
                        hsb = evacp.tile([128, SL], f32, name="hsb")
                        nc.vector.tensor_tensor(out=hsb[:, :], in0=h1sb[:, :],
                                                in1=ysb[:, :], op=ALU.add)
                        nc.sync.dma_start(out=hT_d[:, m, nsl], in_=hsb[:, :])
    import bass_rust as _bass_rust
    _bass_rust.generate_event_semaphores(nc)
    from concourse.library_overlay import lower_extended_insts
    lower_extended_insts(nc)
    return nc


def _get_prog(chunks):
    key = tuple(int(c) for c in chunks)
    if key not in _PROG_CACHE:
        _PROG_CACHE[key] = _build_gnn(key)
    return _PROG_CACHE[key]


def _pack_w(W):
    return np.ascontiguousarray(
        np.asarray(W, np.float32).reshape(2, 128, 2, 128).transpose(1, 0, 2, 3))


def _layer_weights(inp, i):
    import jax
    nw = np.asarray(jax.nn.softmax(inp["na_w"][i]))
    Ws = nw[1] * np.asarray(inp["sage_Wl"][i], np.float32)
    wd = {
        "Wg": nw[0] * np.asarray(inp["gcn_W"][i], np.float32),
        "Ws": Ws,
        "Wsn": -Ws,
        "Wc": nw[3] * np.asarray(inp["gc_Wl"][i], np.float32),
        "Rp": (nw[1] * np.asarray(inp["sage_Wr"][i], np.float32)
               + nw[3] * np.asarray(inp["gc_Wr"][i], np.float32)
               - nw[3] * np.asarray(inp["gc_Wl"][i], np.float32)),
        "W1": np.asarray(inp["gin_W1"][i], np.float32),
        "W2": nw[2] * np.asarray(inp["gin_W2"][i], np.float32),
    }
    b1 = np.asarray(inp["gin_b1"][i], np.float32)
    ball = (nw[0] * np.asarray(inp["gcn_b"][i], np.float32)
            + nw[1] * np.asarray(inp["sage_b"][i], np.float32)
            + nw[2] * np.asarray(inp["gin_b2"][i], np.float32)
            + nw[3] * np.asarray(inp["gc_b"][i], np.float32))
    return wd, b1, ball


def _make_in_maps(x2, wd, b1, ball, idxs_l, colv_l, disv_l, invm_l):
    wpk = {n: _pack_w(wd[n]) for n in wd}
    b1p = np.ascontiguousarray(b1.reshape(2, 128).T)
    ballp = np.ascontiguousarray(ball.reshape(2, 128).T)
    x2c = np.ascontiguousarray(np.asarray(x2, np.float32))
    in_maps = []
    for c in range(NCORES):
        xp = np.zeros((NPAD, D), np.float32)
        xp[:NPC] = x2c[c * NPC:(c + 1) * NPC]
        x2T = np.ascontiguousarray(xp.T.reshape(2, 128, NPAD).transpose(1, 0, 2))
        im = {"x2full": x2c, "x2T": x2T, "invm": invm_l[c],
              "idxs": idxs_l[c], "colv": colv_l[c], "disv": disv_l[c],
              "b1": b1p, "ball": ballp,
              "iotaf": np.ascontiguousarray(
                  np.broadcast_to(np.arange(128, dtype=np.float32), (128, 128)))}
        im.update(wpk)
        in_maps.append(im)
    return in_maps


def _run_layer(nc, in_maps):
    import time
    from concourse.bass_utils import run_bass_kernel_spmd
    t0 = time.monotonic_ns()
    res = run_bass_kernel_spmd(nc, in_maps, list(range(NCORES)), trace=TRACE)
    LAST_WALL_NS.append(time.monotonic_ns() - t0)
    ns = getattr(res, "exec_time_ns", None)
    if ns:
        LAST_EXEC_NS.append(ns)
    h = np.empty((N, D), np.float32)
    for c in range(NCORES):
        hT = np.asarray(res.results[c]["hT"])
        h[c * NPC:(c + 1) * NPC] = hT.transpose(2, 1, 0).reshape(NPAD, D)[:NPC]
    return h


def kernel(**inputs):
    import jax
    import jax.numpy as jnp
    inp = inputs
    cpu = jax.devices("cpu")[0]

    src = np.asarray(inp["edge_index"][0]).astype(np.int64)
    dst = np.asarray(inp["edge_index"][1]).astype(np.int64)
    deg = np.zeros(N, np.float32)
    np.add.at(deg, dst, np.float32(1.0))
    invmax = (1.0 / np.maximum(deg, 1.0)).astype(np.float32)

    with jax.default_device(cpu):
        dis = np.asarray(jax.lax.rsqrt(jnp.asarray(deg) + 1.0))

        # layer 0: all rows of h0 are identical -> single-row compute
        emb0 = np.asarray(inp["emb"])[0]
        h0b = jnp.asarray(np.broadcast_to(emb0, (N, D)))
        sw0 = jax.nn.softmax(inp["se_w"][0, :1], axis=-1)
        fw0 = jax.nn.softmax(inp["fu_w"][0])
        st0 = sw0[0, 1] * h0b
        fused0 = fw0[0] * st0 + fw0[1] * st0 + fw0[2] * st0
        x2_0 = jax.nn.elu(_bn(fused0, inp["bn_gamma"][0], inp["bn_beta"][0]))
        u = np.asarray(x2_0)[0]

        # structured h1: rank-1 + per-degree-class closed form
        nw = jax.nn.softmax(inp["na_w"][0])
        uj = jnp.asarray(u)
        w_gcn = uj @ inp["gcn_W"][0]
        w_sl = uj @ inp["sage_Wl"][0]
        w_sr = uj @ inp["sage_Wr"][0]
        w_gl = uj @ inp["gc_Wl"][0]
        w_gr = uj @ inp["gc_Wr"][0]
        t = np.zeros(N, np.float32)
        np.add.at(t, dst, dis[src].astype(np.float32))
        degs_unique = np.unique(deg)
        uW1 = uj @ inp["gin_W1"][0]
        gin_rows = {}
        for dv in degs_unique:
            hh = (1.0 + np.float32(dv)) * uW1
            gin_rows[float(dv)] = np.asarray(
                jax.nn.relu(hh + inp["gin_b1"][0]) @ inp["gin_W2"][0] + inp["gin_b2"][0])
        gin_tab = np.stack([gin_rows[float(dv)] for dv in degs_unique])
        deg_idx = np.searchsorted(degs_unique, deg)
        gcn_scal = (dis * t + dis * dis).astype(np.float32)
        sage_scal = (deg / np.maximum(deg, 1.0)).astype(np.float32)
        h1 = (nw[0] * (jnp.asarray(gcn_scal)[:, None] * w_gcn[None, :])
              + nw[1] * (jnp.asarray(sage_scal)[:, None] * w_sl[None, :] + w_sr[None, :])
              + nw[2] * jnp.asarray(gin_tab)[jnp.asarray(deg_idx)]
              + nw[3] * (jnp.asarray(deg)[:, None] * w_gl[None, :] + w_gr[None, :]))
        h1 = jnp.asarray(np.asarray(h1, np.float32))

    chunks, idxs_l, colv_l, disv_l = _edge_prep(src, dst, dis)
    nc = _get_prog(chunks)
    invm_l = []
    for c in range(NCORES):
        ivp = np.zeros(NPAD, np.float32)
        ivp[:NPC] = invmax[c * NPC:(c + 1) * NPC]
        invm_l.append(np.ascontiguousarray(np.broadcast_to(ivp, (128, NPAD))))

    h_list = [h0b, h1]
    for i in (1, 2):
        with jax.default_device(cpu):
            sw = jax.nn.softmax(inp["se_w"][i, :i + 1], axis=-1)
            st = jnp.stack([sw[j, 1] * h_list[j] for j in range(i + 1)])
            fw = jax.nn.softmax(inp["fu_w"][i])
            fused = fw[0] * st.sum(0) + fw[1] * st.mean(0) + fw[2] * st.max(0)
            x2 = np.asarray(jax.nn.elu(_bn(fused, inp["bn_gamma"][i], inp["bn_beta"][i])),
                            np.float32)
            wd, b1, ball = _layer_weights(inp, i)
        in_maps = _make_in_maps(x2, wd, b1, ball, idxs_l, colv_l, disv_l, invm_l)
        h = _run_layer(nc, in_maps)
        h_list.append(jnp.asarray(h))

    with jax.default_device(cpu):
        i = 3
        sw = jax.nn.softmax(inp["se_w"][i, :i + 1], axis=-1)
        st = jnp.stack([sw[j, 1] * h_list[j] for j in range(i + 1)])
        fw = jax.nn.softmax(inp["fu_w"][i])
        fused = fw[0] * st.sum(0) + fw[1] * st.mean(0) + fw[2] * st.max(0)
        x2 = jax.nn.elu(_bn(fused, inp["bn_gamma"][i], inp["bn_beta"][i]))
        pooled = jax.ops.segment_sum(x2, jnp.asarray(inp["batch"]), num_segments=NG)
        out = np.asarray(pooled @ inp["cls_W"] + inp["cls_b"], np.float32)
    return out


# revision 27
# speedup vs baseline: 3.1379x; 3.1379x over previous
import numpy as np

N = 30000
E = 480000
D = 256
NG = 256
OUT = 10
NCORES = 8
NPC = N // NCORES          # 3750 nodes per core
NPAD = 3840                # 30 groups of 128
GROUPS = NPAD // 128
WCH = 8                    # gather window (chunks per dma_gather)
NSLICE = 8
SL = NPAD // NSLICE        # 480

TRACE = False
LAST_EXEC_NS = []
LAST_WALL_NS = []
_PROG_CACHE = {}


def _cdiv(a, b):
    return (a + b - 1) // b


def _bn(h, g, b):
    import jax
    m = h.mean(axis=0)
    v = h.var(axis=0)
    return g * (h - m) * jax.lax.rsqrt(v + 1e-5) + b


def _edge_prep(src, dst, dis):
    order = np.argsort(dst, kind="stable")
    ssrc = src[order].astype(np.int64)
    sdst = dst[order].astype(np.int64)
    ents = [[None] * GROUPS for _ in range(NCORES)]
    cnts = np.zeros((NCORES, GROUPS), np.int64)
    for c in range(NCORES):
        for g in range(GROUPS):
            lo = c * NPC + g * 128
            hi = min(lo + 128, (c + 1) * NPC)
            if lo >= hi:
                ents[c][g] = (np.zeros(0, np.int64), np.zeros(0, np.int64), np.zeros(0, np.float32))
                continue
            e_lo = np.searchsorted(sdst, lo, "left")
            e_hi = np.searchsorted(sdst, hi, "left")
            es = ssrc[e_lo:e_hi]
            ed = sdst[e_lo:e_hi]
            selfn = np.arange(lo, hi, dtype=np.int64)
            srcs = np.concatenate([es, selfn])
            cols = np.concatenate([ed - lo, selfn - lo])
            dv = np.concatenate([dis[es] * dis[ed], dis[selfn] * dis[selfn]]).astype(np.float32)
            ents[c][g] = (srcs, cols, dv)
            cnts[c, g] = len(srcs)
    chunks = np.maximum(_cdiv(cnts, 128).max(axis=0), 1).astype(np.int64)
    C = _cdiv(int(chunks.sum()), WCH) * WCH  # pad to full gather windows
    idxs_l, colv_l, disv_l = [], [], []
    for c in range(NCORES):
        gsrc = np.zeros(C * 128, np.int64)
        gcol = np.full(C * 128, -1.0, np.float32)
        gdv = np.zeros(C * 128, np.float32)
        off = 0
        for g in range(GROUPS):
            srcs, cols, dv = ents[c][g]
            n = len(srcs)
            gsrc[off:off + n] = srcs
            gcol[off:off + n] = cols.astype(np.float32)
            gdv[off:off + n] = dv
            off += int(chunks[g]) * 128
        gsrc = (gsrc // NPC) * NPAD + (gsrc % NPC)  # index into all-gathered [8*NPAD, D]
        idx16 = gsrc.astype(np.int16).reshape(C, 8, 16).transpose(2, 0, 1).reshape(16, C * 8)
        idxs_l.append(np.ascontiguousarray(np.tile(idx16, (8, 1))))
        colv_l.append(np.ascontiguousarray(gcol.reshape(C, 128).T))
        disv_l.append(np.ascontiguousarray(gdv.reshape(C, 128).T))
    return chunks, idxs_l, colv_l, disv_l


def _build_gnn(chunks):
    import concourse.bass as bass
    from concourse import mybir, tile
    from concourse.library_config import mlp as _mlp_lib

    C = _cdiv(int(sum(chunks)), WCH) * WCH
    f32 = mybir.dt.float32
    f32r = mybir.dt.float32r
    AF = mybir.ActivationFunctionType
    ALU = mybir.AluOpType

    nc = bass.Bass("TRN2", target_bir_lowering=False, debug=False, num_devices=NCORES)
    x2sh_d = nc.dram_tensor("x2sh", [NPAD, D], f32, kind="ExternalInput").ap()
    x2T_d = nc.dram_tensor("x2T", [128, 2, NPAD], f32, kind="ExternalInput").ap()
    invm_d = nc.dram_tensor("invm", [128, NPAD], f32, kind="ExternalInput").ap()
    idxs_d = nc.dram_tensor("idxs", [128, C * 8], mybir.dt.int16, kind="ExternalInput").ap()
    colv_d = nc.dram_tensor("colv", [128, C], f32, kind="ExternalInput").ap()
    disv_d = nc.dram_tensor("disv", [128, C], f32, kind="ExternalInput").ap()
    wnames = ("Wg", "Ws", "Wsn", "Wc", "Rp", "W1", "W2")
    w_d = {n: nc.dram_tensor(n, [128, 2, 2, 128], f32, kind="ExternalInput").ap() for n in wnames}
    b1_d = nc.dram_tensor("b1", [128, 2], f32, kind="ExternalInput").ap()
    ball_d = nc.dram_tensor("ball", [128, 2], f32, kind="ExternalInput").ap()
    iotaf_d = nc.dram_tensor("iotaf", [128, 128], f32, kind="ExternalInput").ap()
    hT_d = nc.dram_tensor("hT", [128, 2, NPAD], f32, kind="ExternalOutput").ap()

    with tile.TileContext(nc) as tc:
        with tc.tile_pool(name="persist", bufs=1) as pp, \
             tc.tile_pool(name="dramp", bufs=1, space="DRAM") as dp:
            x2T_sb = pp.tile([128, 2, NPAD], f32, tag="x2T", name="x2T_sb")
            PT = pp.tile([128, 2, NPAD], f32, tag="PT", name="PT")
            QT = pp.tile([128, 2, NPAD], f32, tag="QT", name="QT")
            invm_sb = pp.tile([128, NPAD], f32, tag="invm", name="invm_sb")
            idxs_sb = pp.tile([128, C * 8], mybir.dt.int16, tag="idxs", name="idxs_sb")
            colv_sb = pp.tile([128, C], f32, tag="colv", name="colv_sb")
            disv_sb = pp.tile([128, C], f32, tag="disv", name="disv_sb")
            w_sb = {n: pp.tile([128, 2, 2, 128], f32, tag=n, name=n + "_sb") for n in wnames}
            b1_sb = pp.tile([128, 2], f32, tag="b1", name="b1_sb")
            ball_sb = pp.tile([128, 2], f32, tag="ball", name="ball_sb")
            iota_f = pp.tile([128, 128], f32, tag="iota_f", name="iota_f")

            x2b = dp.tile([NPAD, D], f32, tag="x2b", name="x2b")
            x2as = dp.tile([NCORES * NPAD, D], f32, tag="x2as", name="x2as")
            nc.sync.dma_start(out=x2b[:, :], in_=x2sh_d[:, :])
            nc.gpsimd.collective_compute(
                "AllGather", ALU.bypass,
                replica_groups=[list(range(NCORES))],
                ins=[x2b[:, :]], outs=[x2as[:, :]])
            nc.sync.dma_start(out=x2T_sb[:, :, :].bitcast(f32r), in_=x2T_d[:, :, :].bitcast(f32r))
            nc.sync.dma_start(out=invm_sb[:, :], in_=invm_d[:, :])
            nc.sync.dma_start(out=idxs_sb[:, :], in_=idxs_d[:, :])
            nc.sync.dma_start(out=colv_sb[:, :], in_=colv_d[:, :])
            nc.sync.dma_start(out=disv_sb[:, :], in_=disv_d[:, :])
            for n in wnames:
                nc.sync.dma_start(out=w_sb[n][:, :, :, :].bitcast(f32r),
                                  in_=w_d[n][:, :, :, :].bitcast(f32r))
            nc.sync.dma_start(out=b1_sb[:, :], in_=b1_d[:, :])
            nc.sync.dma_start(out=ball_sb[:, :], in_=ball_d[:, :])
            nc.sync.dma_start(out=iota_f[:, :], in_=iotaf_d[:, :])
            nc.gpsimd.load_library(_mlp_lib)
            nidx_reg = nc.gpsimd.to_reg(WCH * 128)

            # ---- aggregation: P' (sum + self) and Q' (sym-norm sum + self) via one-hot matmul
            with tc.tile_pool(name="gat", bufs=2) as gat, \
                 tc.tile_pool(name="ohp", bufs=4) as ohp, \
                 tc.tile_pool(name="aps", bufs=4, space="PSUM") as aps:
                gt = None
                ci = 0
                for g in range(GROUPS):
                    nch = int(chunks[g])
                    ps0 = aps.tile([128, 256], f32, name="ps0")
                    ps1 = aps.tile([128, 256], f32, name="ps1")
                    for j in range(nch):
                        if ci % WCH == 0:
                            gt = gat.tile([128, WCH, 256], f32, name="gt")
                            nc.gpsimd.dma_gather(gt[:, :, :].bitcast(f32r),
                                                 x2as[:, :].bitcast(f32r),
                                                 idxs_sb[:, ci * 8:(ci + WCH) * 8],
                                                 WCH * 128, nidx_reg, 256)
                        slot = ci % WCH
                        oh = ohp.tile([128, 256], f32, name="oh")
                        nc.vector.tensor_tensor(out=oh[:, 0:128].bitcast(f32r),
                                                in0=colv_sb[:, ci:ci + 1].to_broadcast([128, 128])[:],
                                                in1=iota_f[:, :], op=ALU.is_equal)
                        nc.scalar.activation(oh[:, 128:256].bitcast(f32r), oh[:, 0:128], AF.Copy,
                                             bias=0.0, scale=disv_sb[:, ci:ci + 1])
                        nc.tensor.matmul(out=ps0[:, :],
                                         lhsT=gt[:, slot, 0:128].bitcast(f32r),
                                         rhs=oh[:, :].bitcast(f32r),
                                         start=(j == 0), stop=(j == nch - 1),
                                         skip_group_check=True)
                        nc.tensor.matmul(out=ps1[:, :],
                                         lhsT=gt[:, slot, 128:256].bitcast(f32r),
                                         rhs=oh[:, :].bitcast(f32r),
                                         start=(j == 0), stop=(j == nch - 1),
                                         skip_group_check=True)
                        ci += 1
                    gsl = slice(g * 128, (g + 1) * 128)
                    nc.vector.tensor_copy(out=PT[:, 0, gsl].bitcast(f32r), in_=ps0[:, 0:128])
                    nc.scalar.copy(QT[:, 0, gsl].bitcast(f32r), ps0[:, 128:256])
                    nc.vector.tensor_copy(out=PT[:, 1, gsl].bitcast(f32r), in_=ps1[:, 0:128])
                    nc.scalar.copy(QT[:, 1, gsl].bitcast(f32r), ps1[:, 128:256])

            # ---- dense phase: h = Q'Wg + P'Wc + x2 Rp + relu(P'W1+b1)W2 + invm*((P'-x2)Ws) + ball
            with tc.tile_pool(name="tsbp", bufs=2) as tsbp, \
                 tc.tile_pool(name="evac", bufs=4) as evacp, \
                 tc.tile_pool(name="tps", bufs=2, space="PSUM") as tpsp, \
                 tc.tile_pool(name="yps", bufs=2, space="PSUM") as ypsp, \
                 tc.tile_pool(name="hps", bufs=2, space="PSUM") as hpsp:
                for s in range(NSLICE):
                    nsl = slice(s * SL, (s + 1) * SL)
                    tsb = tsbp.tile([128, 2, SL], f32, name="tsb")
                    for m in range(2):
                        tps = tpsp.tile([128, SL], f32, name="tps")
                        for k in range(2):
                            nc.tensor.matmul(out=tps[:, :],
                                             lhsT=w_sb["W1"][:, k, m, :].bitcast(f32r),
                                             rhs=PT[:, k, nsl].bitcast(f32r),
                                             start=(k == 0), stop=(k == 1))
                        nc.scalar.activation(tsb[:, m, :].bitcast(f32r), tps[:, :], AF.Relu,
                                             bias=b1_sb[:, m:m + 1], scale=1.0)
                    for m in range(2):
                        yps = ypsp.tile([128, SL], f32, name="yps")
                        q = 0
                        for k in range(2):
                            for wn, rhs_ap in (("Ws", PT[:, k, nsl]), ("Wsn", x2T_sb[:, k, nsl])):
                                nc.tensor.matmul(out=yps[:, :],
                                                 lhsT=w_sb[wn][:, k, m, :].bitcast(f32r),
                                                 rhs=rhs_ap.bitcast(f32r),
                                                 start=(q == 0), stop=(q == 3))
                                q += 1
                        hps = hpsp.tile([128, SL], f32, name="hps")
                        q = 0
                        for k in range(2):
                            for wn, rhs_ap in (("Wg", QT[:, k, nsl]), ("Wc", PT[:, k, nsl]),
                                               ("Rp", x2T_sb[:, k, nsl]), ("W2", tsb[:, k, :])):
                                nc.tensor.matmul(out=hps[:, :],
                                                 lhsT=w_sb[wn][:, k, m, :].bitcast(f32r),
                                                 rhs=rhs_ap.bitcast(f32r),
                                                 start=(q == 0), stop=(q == 7))
                                q += 1
                        ysb = evacp.tile([128, SL], f32, name="ysb")
                        nc.vector.tensor_tensor(out=ysb[:, :], in0=yps[:, :],
                                                in1=invm_sb[:, nsl], op=ALU.mult)
                        h1sb = evacp.tile([128, SL], f32, name="h1sb")
                        nc.scalar.activation(h1sb[:, :], hps[:, :], AF.Identity,
                                             bias=ball_sb[:, m:m + 1], scale=1.0)
                        hsb = evacp.tile([128, SL], f32, name="hsb")
                        nc.vector.tensor_tensor(out=hsb[:, :], in0=h1sb[:, :],
                                                in1=ysb[:, :], op=ALU.add)
                        nc.sync.dma_start(out=hT_d[:, m, nsl], in_=hsb[:, :])
    import bass_rust as _bass_rust
    _bass_rust.generate_event_semaphores(nc)
    from concourse.library_overlay import lower_extended_insts
    lower_extended_insts(nc)
    return nc


def _get_prog(chunks):
    key = tuple(int(c) for c in chunks)
    if key not in _PROG_CACHE:
        _PROG_CACHE[key] = _build_gnn(key)
    return _PROG_CACHE[key]


def _pack_w(W):
    return np.ascontiguousarray(
        np.asarray(W, np.float32).reshape(2, 128, 2, 128).transpose(1, 0, 2, 3))


def _layer_weights(inp, i):
    import jax
    nw = np.asarray(jax.nn.softmax(inp["na_w"][i]))
    Ws = nw[1] * np.asarray(inp["sage_Wl"][i], np.float32)
    wd = {
        "Wg": nw[0] * np.asarray(inp["gcn_W"][i], np.float32),
        "Ws": Ws,
        "Wsn": -Ws,
        "Wc": nw[3] * np.asarray(inp["gc_Wl"][i], np.float32),
        "Rp": (nw[1] * np.asarray(inp["sage_Wr"][i], np.float32)
               + nw[3] * np.asarray(inp["gc_Wr"][i], np.float32)
               - nw[3] * np.asarray(inp["gc_Wl"][i], np.float32)),
        "W1": np.asarray(inp["gin_W1"][i], np.float32),
        "W2": nw[2] * np.asarray(inp["gin_W2"][i], np.float32),
    }
    b1 = np.asarray(inp["gin_b1"][i], np.float32)
    ball = (nw[0] * np.asarray(inp["gcn_b"][i], np.float32)
            + nw[1] * np.asarray(inp["sage_b"][i], np.float32)
            + nw[2] * np.asarray(inp["gin_b2"][i], np.float32)
            + nw[3] * np.asarray(inp["gc_b"][i], np.float32))
    return wd, b1, ball


def _make_in_maps(x2, wd, b1, ball, idxs_l, colv_l, disv_l, invm_l):
    wpk = {n: _pack_w(wd[n]) for n in wd}
    b1p = np.ascontiguousarray(b1.reshape(2, 128).T)
    ballp = np.ascontiguousarray(ball.reshape(2, 128).T)
    x2c = np.ascontiguousarray(np.asarray(x2, np.float32))
    in_maps = []
    for c in range(NCORES):
        xp = np.zeros((NPAD, D), np.float32)
        xp[:NPC] = x2c[c * NPC:(c + 1) * NPC]
        x2T = np.ascontiguousarray(xp.T.reshape(2, 128, NPAD).transpose(1, 0, 2))
        im = {"x2sh": xp, "x2T": x2T, "invm": invm_l[c],
              "idxs": idxs_l[c], "colv": colv_l[c], "disv": disv_l[c],
              "b1": b1p, "ball": ballp,
              "iotaf": np.ascontiguousarray(
                  np.broadcast_to(np.arange(128, dtype=np.float32), (128, 128)))}
        im.update(wpk)
        in_maps.append(im)
    return in_maps


def _run_layer(nc, in_maps):
    import time
    from concourse.bass_utils import run_bass_kernel_spmd
    t0 = time.monotonic_ns()
    res = run_bass_kernel_spmd(nc, in_maps, list(range(NCORES)), trace=TRACE)
    LAST_WALL_NS.append(time.monotonic_ns() - t0)
    ns = getattr(res, "exec_time_ns", None)
    if ns:
        LAST_EXEC_NS.append(ns)
    h = np.empty((N, D), np.float32)
    for c in range(NCORES):
        hT = np.asarray(res.results[c]["hT"])
        h[c * NPC:(c + 1) * NPC] = hT.transpose(2, 1, 0).reshape(NPAD, D)[:NPC]
    return h


def kernel(**inputs):
    import jax
    import jax.numpy as jnp
    inp = inputs
    cpu = jax.devices("cpu")[0]

    src = np.asarray(inp["edge_index"][0]).astype(np.int64)
    dst = np.asarray(inp["edge_index"][1]).astype(np.int64)
    deg = np.zeros(N, np.float32)
    np.add.at(deg, dst, np.float32(1.0))
    invmax = (1.0 / np.maximum(deg, 1.0)).astype(np.float32)

    with jax.default_device(cpu):
        dis = np.asarray(jax.lax.rsqrt(jnp.asarray(deg) + 1.0))

        # layer 0: all rows of h0 are identical -> single-row compute
        emb0 = np.asarray(inp["emb"])[0]
        h0b = jnp.asarray(np.broadcast_to(emb0, (N, D)))
        sw0 = jax.nn.softmax(inp["se_w"][0, :1], axis=-1)
        fw0 = jax.nn.softmax(inp["fu_w"][0])
        st0 = sw0[0, 1] * h0b
        fused0 = fw0[0] * st0 + fw0[1] * st0 + fw0[2] * st0
        x2_0 = jax.nn.elu(_bn(fused0, inp["bn_gamma"][0], inp["bn_beta"][0]))
        u = np.asarray(x2_0)[0]

        # structured h1: rank-1 + per-degree-class closed form
        nw = jax.nn.softmax(inp["na_w"][0])
        uj = jnp.asarray(u)
        w_gcn = uj @ inp["gcn_W"][0]
        w_sl = uj @ inp["sage_Wl"][0]
        w_sr = uj @ inp["sage_Wr"][0]
        w_gl = uj @ inp["gc_Wl"][0]
        w_gr = uj @ inp["gc_Wr"][0]
        t = np.zeros(N, np.float32)
        np.add.at(t, dst, dis[src].astype(np.float32))
        degs_unique = np.unique(deg)
        uW1 = uj @ inp["gin_W1"][0]
        gin_rows = {}
        for dv in degs_unique:
            hh = (1.0 + np.float32(dv)) * uW1
            gin_rows[float(dv)] = np.asarray(
                jax.nn.relu(hh + inp["gin_b1"][0]) @ inp["gin_W2"][0] + inp["gin_b2"][0])
        gin_tab = np.stack([gin_rows[float(dv)] for dv in degs_unique])
        deg_idx = np.searchsorted(degs_unique, deg)
        gcn_scal = (dis * t + dis * dis).astype(np.float32)
        sage_scal = (deg / np.maximum(deg, 1.0)).astype(np.float32)
        h1 = (nw[0] * (jnp.asarray(gcn_scal)[:, None] * w_gcn[None, :])
              + nw[1] * (jnp.asarray(sage_scal)[:, None] * w_sl[None, :] + w_sr[None, :])
              + nw[2] * jnp.asarray(gin_tab)[jnp.asarray(deg_idx)]
              + nw[3] * (jnp.asarray(deg)[:, None] * w_gl[None, :] + w_gr[None, :]))
        h1 = jnp.asarray(np.asarray(h1, np.float32))

    chunks, idxs_l, colv_l, disv_l = _edge_prep(src, dst, dis)
    nc = _get_prog(chunks)
    invm_l = []
    for c in range(NCORES):
        ivp = np.zeros(NPAD, np.float32)
        ivp[:NPC] = invmax[c * NPC:(c + 1) * NPC]
        invm_l.append(np.ascontiguousarray(np.broadcast_to(ivp, (128, NPAD))))

    h_list = [h0b, h1]
    for i in (1, 2):
        with jax.default_device(cpu):
            sw = jax.nn.softmax(inp["se_w"][i, :i + 1], axis=-1)
            st = jnp.stack([sw[j, 1] * h_list[j] for j in range(i + 1)])
            fw = jax.nn.softmax(inp["fu_w"][i])
            fused = fw[0] * st.sum(0) + fw[1] * st.mean(0) + fw[2] * st.max(0)
            x2 = np.asarray(jax.nn.elu(_bn(fused, inp["bn_gamma"][i], inp["bn_beta"][i])),
                            np.float32)
            wd, b1, ball = _layer_weights(inp, i)
        in_maps = _make_in_maps(x2, wd, b1, ball, idxs_l, colv_l, disv_l, invm_l)
        h = _run_layer(nc, in_maps)
        h_list.append(jnp.asarray(h))

    with jax.default_device(cpu):
        i = 3
        sw = jax.nn.softmax(inp["se_w"][i, :i + 1], axis=-1)
        st = jnp.stack([sw[j, 1] * h_list[j] for j in range(i + 1)])
        fw = jax.nn.softmax(inp["fu_w"][i])
        fused = fw[0] * st.sum(0) + fw[1] * st.mean(0) + fw[2] * st.max(0)
        x2 = jax.nn.elu(_bn(fused, inp["bn_gamma"][i], inp["bn_beta"][i]))
        pooled = jax.ops.segment_sum(x2, jnp.asarray(inp["batch"]), num_segments=NG)
        out = np.asarray(pooled @ inp["cls_W"] + inp["cls_b"], np.float32)
    return out


# revision 37
# speedup vs baseline: 5.0255x; 1.6015x over previous
import numpy as np

N = 30000
E = 480000
D = 256
NG = 256
OUT = 10
NCORES = 8
NPC = N // NCORES          # 3750 nodes per core
NPAD = 3840                # 30 groups of 128
GROUPS = NPAD // 128
WCH = 8                    # gather window (chunks per dma_gather)
NSLICE = 8
SL = NPAD // NSLICE        # 480

TRACE = False
LAST_EXEC_NS = []
LAST_WALL_NS = []
_PROG_CACHE = {}


def _cdiv(a, b):
    return (a + b - 1) // b


def _bn(h, g, b):
    import jax
    m = h.mean(axis=0)
    v = h.var(axis=0)
    return g * (h - m) * jax.lax.rsqrt(v + 1e-5) + b


def _edge_prep(src, dst, dis):
    order = np.argsort(dst, kind="stable")
    ssrc = src[order].astype(np.int64)
    sdst = dst[order].astype(np.int64)
    ents = [[None] * GROUPS for _ in range(NCORES)]
    cnts = np.zeros((NCORES, GROUPS), np.int64)
    for c in range(NCORES):
        for g in range(GROUPS):
            lo = c * NPC + g * 128
            hi = min(lo + 128, (c + 1) * NPC)
            if lo >= hi:
                ents[c][g] = (np.zeros(0, np.int64), np.zeros(0, np.int64), np.zeros(0, np.float32))
                continue
            e_lo = np.searchsorted(sdst, lo, "left")
            e_hi = np.searchsorted(sdst, hi, "left")
            es = ssrc[e_lo:e_hi]
            ed = sdst[e_lo:e_hi]
            selfn = np.arange(lo, hi, dtype=np.int64)
            srcs = np.concatenate([es, selfn])
            cols = np.concatenate([ed - lo, selfn - lo])
            dv = np.concatenate([dis[es] * dis[ed], dis[selfn] * dis[selfn]]).astype(np.float32)
            ents[c][g] = (srcs, cols, dv)
            cnts[c, g] = len(srcs)
    chunks = np.maximum(_cdiv(cnts, 128).max(axis=0), 1).astype(np.int64)
    C = _cdiv(int(chunks.sum()), WCH) * WCH  # pad to full gather windows
    idxs_l, colv_l, disv_l = [], [], []
    for c in range(NCORES):
        gsrc = np.zeros(C * 128, np.int64)
        gcol = np.full(C * 128, -1.0, np.float32)
        gdv = np.zeros(C * 128, np.float32)
        off = 0
        for g in range(GROUPS):
            srcs, cols, dv = ents[c][g]
            n = len(srcs)
            gsrc[off:off + n] = srcs
            gcol[off:off + n] = cols.astype(np.float32)
            gdv[off:off + n] = dv
            off += int(chunks[g]) * 128
        gsrc = (gsrc // NPC) * NPAD + (gsrc % NPC)  # index into all-gathered [8*NPAD, D]
        idx16 = gsrc.astype(np.int16).reshape(C, 8, 16).transpose(2, 0, 1).reshape(16, C * 8)
        idxs_l.append(np.ascontiguousarray(idx16))
        colv_l.append(np.ascontiguousarray(gcol.reshape(C, 128).T))
        disv_l.append(np.ascontiguousarray(gdv.reshape(C, 128).T))
    return chunks, idxs_l, colv_l, disv_l


def _build_gnn(chunks):
    import concourse.bass as bass
    from concourse import mybir, tile
    from concourse.library_config import mlp as _mlp_lib

    C = _cdiv(int(sum(chunks)), WCH) * WCH
    f32 = mybir.dt.float32
    f32r = mybir.dt.float32r
    AF = mybir.ActivationFunctionType
    ALU = mybir.AluOpType

    nc = bass.Bass("TRN2", target_bir_lowering=False, debug=False, num_devices=NCORES)
    x2sh_d = nc.dram_tensor("x2sh", [NPAD, D], f32, kind="ExternalInput").ap()
    invm_d = nc.dram_tensor("invm", [128, NPAD], f32, kind="ExternalInput").ap()
    idxs_d = nc.dram_tensor("idxs", [16, C * 8], mybir.dt.int16, kind="ExternalInput").ap()
    colv_d = nc.dram_tensor("colv", [128, C], f32, kind="ExternalInput").ap()
    disv_d = nc.dram_tensor("disv", [128, C], f32, kind="ExternalInput").ap()
    wnames = ("Wg", "Ws", "Wsn", "Wc", "Rp", "W1", "W2")
    w_d = {n: nc.dram_tensor(n, [128, 2, 2, 128], f32, kind="ExternalInput").ap() for n in wnames}
    b1_d = nc.dram_tensor("b1", [128, 2], f32, kind="ExternalInput").ap()
    ball_d = nc.dram_tensor("ball", [128, 2], f32, kind="ExternalInput").ap()
    iotaf_d = nc.dram_tensor("iotaf", [128, 128], f32, kind="ExternalInput").ap()
    idn_d = nc.dram_tensor("idn", [128, 128], f32, kind="ExternalInput").ap()
    hT_d = nc.dram_tensor("hT", [128, 2, NPAD], f32, kind="ExternalOutput").ap()

    with tile.TileContext(nc) as tc:
        with tc.tile_pool(name="persist", bufs=1) as pp, \
             tc.tile_pool(name="dramp", bufs=1, space="DRAM") as dp:
            x2T_sb = pp.tile([128, 2, NPAD], f32, tag="x2T", name="x2T_sb")
            PT = pp.tile([128, 2, NPAD], f32, tag="PT", name="PT")
            QT = pp.tile([128, 2, NPAD], f32, tag="QT", name="QT")
            invm_sb = pp.tile([128, NPAD], f32, tag="invm", name="invm_sb")
            idxs_sb = pp.tile([128, C * 8], mybir.dt.int16, tag="idxs", name="idxs_sb")
            colv_sb = pp.tile([128, C], f32, tag="colv", name="colv_sb")
            disv_sb = pp.tile([128, C], f32, tag="disv", name="disv_sb")
            w_sb = {n: pp.tile([128, 2, 2, 128], f32, tag=n, name=n + "_sb") for n in wnames}
            b1_sb = pp.tile([128, 2], f32, tag="b1", name="b1_sb")
            ball_sb = pp.tile([128, 2], f32, tag="ball", name="ball_sb")
            iota_f = pp.tile([128, 128], f32, tag="iota_f", name="iota_f")
            idn_sb = pp.tile([128, 128], f32, tag="idn", name="idn_sb")

            x2b = dp.tile([NPAD, D], f32, tag="x2b", name="x2b")
            x2as = dp.tile([NCORES * NPAD, D], f32, tag="x2as", name="x2as")
            nc.sync.dma_start(out=x2b[:, :], in_=x2sh_d[:, :])
            nc.gpsimd.collective_compute(
                "AllGather", ALU.bypass,
                replica_groups=[list(range(NCORES))],
                ins=[x2b[:, :]], outs=[x2as[:, :]])
            nc.sync.dma_start(out=idn_sb[:, :].bitcast(f32r),
                              in_=idn_d[:, :].bitcast(f32r))
            with tc.tile_pool(name="xck", bufs=3) as xckp, \
                 tc.tile_pool(name="tpp", bufs=3, space="PSUM") as tppp:
                for g in range(GROUPS):
                    xck = xckp.tile([128, D], f32, name="xck")
                    nc.sync.dma_start(out=xck[:, :].bitcast(f32r),
                                      in_=x2sh_d[g * 128:(g + 1) * 128, :].bitcast(f32r))
                    for k in range(2):
                        tp = tppp.tile([128, 128], f32, name="tp")
                        nc.tensor.transpose(tp[:, :].bitcast(f32r),
                                            xck[:, k * 128:(k + 1) * 128].bitcast(f32r),
                                            idn_sb[:, :].bitcast(f32r))
                        nc.scalar.copy(x2T_sb[:, k, g * 128:(g + 1) * 128].bitcast(f32r),
                                       tp[:, :])
            nc.sync.dma_start(out=invm_sb[:, :], in_=invm_d[:, :])
            for k in range(8):
                nc.sync.dma_start(out=idxs_sb[16 * k:16 * (k + 1), :], in_=idxs_d[:, :])
            nc.sync.dma_start(out=colv_sb[:, :], in_=colv_d[:, :])
            nc.sync.dma_start(out=disv_sb[:, :], in_=disv_d[:, :])
            for n in wnames:
                nc.sync.dma_start(out=w_sb[n][:, :, :, :].bitcast(f32r),
                                  in_=w_d[n][:, :, :, :].bitcast(f32r))
            nc.sync.dma_start(out=b1_sb[:, :], in_=b1_d[:, :])
            nc.sync.dma_start(out=ball_sb[:, :], in_=ball_d[:, :])
            nc.sync.dma_start(out=iota_f[:, :], in_=iotaf_d[:, :])
            nc.gpsimd.load_library(_mlp_lib)
            nidx_reg = nc.gpsimd.to_reg(WCH * 128)

            # ---- aggregation: P' (sum + self) and Q' (sym-norm sum + self) via one-hot matmul
            with tc.tile_pool(name="gat", bufs=2) as gat, \
                 tc.tile_pool(name="ohp", bufs=4) as ohp, \
                 tc.tile_pool(name="aps", bufs=4, space="PSUM") as aps:
                gt = None
                ci = 0
                for g in range(GROUPS):
                    nch = int(chunks[g])
                    ps0 = aps.tile([128, 256], f32, name="ps0")
                    ps1 = aps.tile([128, 256], f32, name="ps1")
                    for j in range(nch):
                        if ci % WCH == 0:
                            gt = gat.tile([128, WCH, 256], f32, name="gt")
                            nc.gpsimd.dma_gather(gt[:, :, :].bitcast(f32r),
                                                 x2as[:, :].bitcast(f32r),
                                                 idxs_sb[:, ci * 8:(ci + WCH) * 8],
                                                 WCH * 128, nidx_reg, 256)
                        slot = ci % WCH
                        oh = ohp.tile([128, 256], f32, name="oh")
                        nc.vector.tensor_tensor(out=oh[:, 0:128].bitcast(f32r),
                                                in0=colv_sb[:, ci:ci + 1].to_broadcast([128, 128])[:],
                                                in1=iota_f[:, :], op=ALU.is_equal)
                        nc.scalar.activation(oh[:, 128:256].bitcast(f32r), oh[:, 0:128], AF.Copy,
                                             bias=0.0, scale=disv_sb[:, ci:ci + 1])
                        nc.tensor.matmul(out=ps0[:, :],
                                         lhsT=gt[:, slot, 0:128].bitcast(f32r),
                                         rhs=oh[:, :].bitcast(f32r),
                                         start=(j == 0), stop=(j == nch - 1),
                                         skip_group_check=True)
                        nc.tensor.matmul(out=ps1[:, :],
                                         lhsT=gt[:, slot, 128:256].bitcast(f32r),
                                         rhs=oh[:, :].bitcast(f32r),
                                         start=(j == 0), stop=(j == nch - 1),
                                         skip_group_check=True)
                        ci += 1
                    gsl = slice(g * 128, (g + 1) * 128)
                    nc.vector.tensor_copy(out=PT[:, 0, gsl].bitcast(f32r), in_=ps0[:, 0:128])
                    nc.scalar.copy(QT[:, 0, gsl].bitcast(f32r), ps0[:, 128:256])
                    nc.vector.tensor_copy(out=PT[:, 1, gsl].bitcast(f32r), in_=ps1[:, 0:128])
                    nc.scalar.copy(QT[:, 1, gsl].bitcast(f32r), ps1[:, 128:256])

            # ---- dense phase: h = Q'Wg + P'Wc + x2 Rp + relu(P'W1+b1)W2 + invm*((P'-x2)Ws) + ball
            with tc.tile_pool(name="tsbp", bufs=2) as tsbp, \
                 tc.tile_pool(name="evac", bufs=4) as evacp, \
                 tc.tile_pool(name="tps", bufs=2, space="PSUM") as tpsp, \
                 tc.tile_pool(name="yps", bufs=2, space="PSUM") as ypsp, \
                 tc.tile_pool(name="hps", bufs=2, space="PSUM") as hpsp:
                for s in range(NSLICE):
                    nsl = slice(s * SL, (s + 1) * SL)
                    tsb = tsbp.tile([128, 2, SL], f32, name="tsb")
                    for m in range(2):
                        tps = tpsp.tile([128, SL], f32, name="tps")
                        for k in range(2):
                            nc.tensor.matmul(out=tps[:, :],
                                             lhsT=w_sb["W1"][:, k, m, :].bitcast(f32r),
                                             rhs=PT[:, k, nsl].bitcast(f32r),
                                             start=(k == 0), stop=(k == 1))
                        nc.scalar.activation(tsb[:, m, :].bitcast(f32r), tps[:, :], AF.Relu,
                                             bias=b1_sb[:, m:m + 1], scale=1.0)
                    for m in range(2):
                        yps = ypsp.tile([128, SL], f32, name="yps")
                        q = 0
                        for k in range(2):
                            for wn, rhs_ap in (("Ws", PT[:, k, nsl]), ("Wsn", x2T_sb[:, k, nsl])):
                                nc.tensor.matmul(out=yps[:, :],
                                                 lhsT=w_sb[wn][:, k, m, :].bitcast(f32r),
                                                 rhs=rhs_ap.bitcast(f32r),
                                                 start=(q == 0), stop=(q == 3))
                                q += 1
                        hps = hpsp.tile([128, SL], f32, name="hps")
                        q = 0
                        for k in range(2):
                            for wn, rhs_ap in (("Wg", QT[:, k, nsl]), ("Wc", PT[:, k, nsl]),
                                               ("Rp", x2T_sb[:, k, nsl]), ("W2", tsb[:, k, :])):
                                nc.tensor.matmul(out=hps[:, :],
                                                 lhsT=w_sb[wn][:, k, m, :].bitcast(f32r),
                                                 rhs=rhs_ap.bitcast(f32r),
                                                 start=(q == 0), stop=(q == 7))
                                q += 1
                        ysb = evacp.tile([128, SL], f32, name="ysb")
                        nc.vector.tensor_tensor(out=ysb[:, :], in0=yps[:, :],
                                                in1=invm_sb[:, nsl], op=ALU.mult)
                        h1sb = evacp.tile([128, SL], f32, name="h1sb")
                        nc.scalar.activation(h1sb[:, :], hps[:, :], AF.Identity,
                                             bias=ball_sb[:, m:m + 1], scale=1.0)
                        hsb = evacp.tile([128, SL], f32, name="hsb")
                        nc.vector.tensor_tensor(out=hsb[:, :], in0=h1sb[:, :],
                                                in1=ysb[:, :], op=ALU.add)
                        nc.sync.dma_start(out=hT_d[:, m, nsl], in_=hsb[:, :])
    import bass_rust as _bass_rust
    _bass_rust.generate_event_semaphores(nc)
    from concourse.library_overlay import lower_extended_insts
    lower_extended_insts(nc)
    return nc


def _get_prog(chunks):
    key = tuple(int(c) for c in chunks)
    if key not in _PROG_CACHE:
        _PROG_CACHE[key] = _build_gnn(key)
    return _PROG_CACHE[key]


def _pack_w(W):
    return np.ascontiguousarray(
        np.asarray(W, np.float32).reshape(2, 128, 2, 128).transpose(1, 0, 2, 3))


def _layer_weights(inp, i):
    import jax
    nw = np.asarray(jax.nn.softmax(inp["na_w"][i]))
    Ws = nw[1] * np.asarray(inp["sage_Wl"][i], np.float32)
    wd = {
        "Wg": nw[0] * np.asarray(inp["gcn_W"][i], np.float32),
        "Ws": Ws,
        "Wsn": -Ws,
        "Wc": nw[3] * np.asarray(inp["gc_Wl"][i], np.float32),
        "Rp": (nw[1] * np.asarray(inp["sage_Wr"][i], np.float32)
               + nw[3] * np.asarray(inp["gc_Wr"][i], np.float32)
               - nw[3] * np.asarray(inp["gc_Wl"][i], np.float32)),
        "W1": np.asarray(inp["gin_W1"][i], np.float32),
        "W2": nw[2] * np.asarray(inp["gin_W2"][i], np.float32),
    }
    b1 = np.asarray(inp["gin_b1"][i], np.float32)
    ball = (nw[0] * np.asarray(inp["gcn_b"][i], np.float32)
            + nw[1] * np.asarray(inp["sage_b"][i], np.float32)
            + nw[2] * np.asarray(inp["gin_b2"][i], np.float32)
            + nw[3] * np.asarray(inp["gc_b"][i], np.float32))
    return wd, b1, ball


def _make_in_maps(x2, wd, b1, ball, idxs_l, colv_l, disv_l, invm_l):
    wpk = {n: _pack_w(wd[n]) for n in wd}
    b1p = np.ascontiguousarray(b1.reshape(2, 128).T)
    ballp = np.ascontiguousarray(ball.reshape(2, 128).T)
    x2c = np.ascontiguousarray(np.asarray(x2, np.float32))
    in_maps = []
    for c in range(NCORES):
        xp = np.zeros((NPAD, D), np.float32)
        xp[:NPC] = x2c[c * NPC:(c + 1) * NPC]
        im = {"x2sh": xp, "invm": invm_l[c],
              "idxs": idxs_l[c], "colv": colv_l[c], "disv": disv_l[c],
              "b1": b1p, "ball": ballp,
              "iotaf": np.ascontiguousarray(
                  np.broadcast_to(np.arange(128, dtype=np.float32), (128, 128))),
              "idn": np.eye(128, dtype=np.float32)}
        im.update(wpk)
        in_maps.append(im)
    return in_maps


def _run_layer(nc, in_maps):
    import time
    from concourse.bass_utils import run_bass_kernel_spmd
    t0 = time.monotonic_ns()
    res = run_bass_kernel_spmd(nc, in_maps, list(range(NCORES)), trace=TRACE)
    LAST_WALL_NS.append(time.monotonic_ns() - t0)
    ns = getattr(res, "exec_time_ns", None)
    if ns:
        LAST_EXEC_NS.append(ns)
    h = np.empty((N, D), np.float32)
    for c in range(NCORES):
        hT = np.asarray(res.results[c]["hT"])
        h[c * NPC:(c + 1) * NPC] = hT.transpose(2, 1, 0).reshape(NPAD, D)[:NPC]
    return h


def kernel(**inputs):
    import jax
    import jax.numpy as jnp
    inp = inputs
    cpu = jax.devices("cpu")[0]

    src = np.asarray(inp["edge_index"][0]).astype(np.int64)
    dst = np.asarray(inp["edge_index"][1]).astype(np.int64)
    deg = np.zeros(N, np.float32)
    np.add.at(deg, dst, np.float32(1.0))
    invmax = (1.0 / np.maximum(deg, 1.0)).astype(np.float32)

    with jax.default_device(cpu):
        dis = np.asarray(jax.lax.rsqrt(jnp.asarray(deg) + 1.0))

        # layer 0: all rows of h0 are identical -> single-row compute
        emb0 = np.asarray(inp["emb"])[0]
        h0b = jnp.asarray(np.broadcast_to(emb0, (N, D)))
        sw0 = jax.nn.softmax(inp["se_w"][0, :1], axis=-1)
        fw0 = jax.nn.softmax(inp["fu_w"][0])
        st0 = sw0[0, 1] * h0b
        fused0 = fw0[0] * st0 + fw0[1] * st0 + fw0[2] * st0
        x2_0 = jax.nn.elu(_bn(fused0, inp["bn_gamma"][0], inp["bn_beta"][0]))
        u = np.asarray(x2_0)[0]

        # structured h1: rank-1 + per-degree-class closed form
        nw = jax.nn.softmax(inp["na_w"][0])
        uj = jnp.asarray(u)
        w_gcn = uj @ inp["gcn_W"][0]
        w_sl = uj @ inp["sage_Wl"][0]
        w_sr = uj @ inp["sage_Wr"][0]
        w_gl = uj @ inp["gc_Wl"][0]
        w_gr = uj @ inp["gc_Wr"][0]
        t = np.zeros(N, np.float32)
        np.add.at(t, dst, dis[src].astype(np.float32))
        degs_unique = np.unique(deg)
        uW1 = uj @ inp["gin_W1"][0]
        gin_rows = {}
        for dv in degs_unique:
            hh = (1.0 + np.float32(dv)) * uW1
            gin_rows[float(dv)] = np.asarray(
                jax.nn.relu(hh + inp["gin_b1"][0]) @ inp["gin_W2"][0] + inp["gin_b2"][0])
        gin_tab = np.stack([gin_rows[float(dv)] for dv in degs_unique])
        deg_idx = np.searchsorted(degs_unique, deg)
        gcn_scal = (dis * t + dis * dis).astype(np.float32)
        sage_scal = (deg / np.maximum(deg, 1.0)).astype(np.float32)
        h1 = (nw[0] * (jnp.asarray(gcn_scal)[:, None] * w_gcn[None, :])
              + nw[1] * (jnp.asarray(sage_scal)[:, None] * w_sl[None, :] + w_sr[None, :])
              + nw[2] * jnp.asarray(gin_tab)[jnp.asarray(deg_idx)]
              + nw[3] * (jnp.asarray(deg)[:, None] * w_gl[None, :] + w_gr[None, :]))
        h1 = jnp.asarray(np.asarray(h1, np.float32))

    chunks, idxs_l, colv_l, disv_l = _edge_prep(src, dst, dis)
    nc = _get_prog(chunks)
    invm_l = []
    for c in range(NCORES):
        ivp = np.zeros(NPAD, np.float32)
        ivp[:NPC] = invmax[c * NPC:(c + 1) * NPC]
        invm_l.append(np.ascontiguousarray(np.broadcast_to(ivp, (128, NPAD))))

    h_list = [h0b, h1]
    for i in (1, 2):
        with jax.default_device(cpu):
            sw = jax.nn.softmax(inp["se_w"][i, :i + 1], axis=-1)
            st = jnp.stack([sw[j, 1] * h_list[j] for j in range(i + 1)])
            fw = jax.nn.softmax(inp["fu_w"][i])
            fused = fw[0] * st.sum(0) + fw[1] * st.mean(0) + fw[2] * st.max(0)
            x2 = np.asarray(jax.nn.elu(_bn(fused, inp["bn_gamma"][i], inp["bn_beta"][i])),
                            np.float32)
            wd, b1, ball = _layer_weights(inp, i)
        in_maps = _make_in_maps(x2, wd, b1, ball, idxs_l, colv_l, disv_l, invm_l)
        h = _run_layer(nc, in_maps)
        h_list.append(jnp.asarray(h))

    with jax.default_device(cpu):
        i = 3
        sw = jax.nn.softmax(inp["se_w"][i, :i + 1], axis=-1)
        st = jnp.stack([sw[j, 1] * h_list[j] for j in range(i + 1)])
        fw = jax.nn.softmax(inp["fu_w"][i])
        fused = fw[0] * st.sum(0) + fw[1] * st.mean(0) + fw[2] * st.max(0)
        x2 = jax.nn.elu(_bn(fused, inp["bn_gamma"][i], inp["bn_beta"][i]))
        pooled = jax.ops.segment_sum(x2, jnp.asarray(inp["batch"]), num_segments=NG)
        out = np.asarray(pooled @ inp["cls_W"] + inp["cls_b"], np.float32)
    return out


# revision 41
# speedup vs baseline: 5.5118x; 1.0968x over previous
import numpy as np

N = 30000
E = 480000
D = 256
NG = 256
OUT = 10
NCORES = 8
NPC = N // NCORES          # 3750 nodes per core
NPAD = 3840                # 30 groups of 128
GROUPS = NPAD // 128
WCH = 8                    # gather window (chunks per dma_gather)
NSLICE = 8
SL = NPAD // NSLICE        # 480

TRACE = False
LAST_EXEC_NS = []
LAST_WALL_NS = []
_PROG_CACHE = {}


def _cdiv(a, b):
    return (a + b - 1) // b


def _bn(h, g, b):
    import jax
    m = h.mean(axis=0)
    v = h.var(axis=0)
    return g * (h - m) * jax.lax.rsqrt(v + 1e-5) + b


def _edge_prep(src, dst, dis):
    order = np.argsort(dst, kind="stable")
    ssrc = src[order].astype(np.int64)
    sdst = dst[order].astype(np.int64)
    ents = [[None] * GROUPS for _ in range(NCORES)]
    cnts = np.zeros((NCORES, GROUPS), np.int64)
    for c in range(NCORES):
        for g in range(GROUPS):
            lo = c * NPC + g * 128
            hi = min(lo + 128, (c + 1) * NPC)
            if lo >= hi:
                ents[c][g] = (np.zeros(0, np.int64), np.zeros(0, np.int64), np.zeros(0, np.float32))
                continue
            e_lo = np.searchsorted(sdst, lo, "left")
            e_hi = np.searchsorted(sdst, hi, "left")
            es = ssrc[e_lo:e_hi]
            ed = sdst[e_lo:e_hi]
            selfn = np.arange(lo, hi, dtype=np.int64)
            srcs = np.concatenate([es, selfn])
            cols = np.concatenate([ed - lo, selfn - lo])
            dv = np.concatenate([dis[es] * dis[ed], dis[selfn] * dis[selfn]]).astype(np.float32)
            ents[c][g] = (srcs, cols, dv)
            cnts[c, g] = len(srcs)
    chunks = np.maximum(_cdiv(cnts, 128).max(axis=0), 1).astype(np.int64)
    C = _cdiv(int(chunks.sum()), WCH) * WCH  # pad to full gather windows
    idxs_l, colv_l, disv_l = [], [], []
    for c in range(NCORES):
        gsrc = np.zeros(C * 128, np.int64)
        gcol = np.full(C * 128, -1.0, np.float32)
        gdv = np.zeros(C * 128, np.float32)
        off = 0
        for g in range(GROUPS):
            srcs, cols, dv = ents[c][g]
            n = len(srcs)
            gsrc[off:off + n] = srcs
            gcol[off:off + n] = cols.astype(np.float32)
            gdv[off:off + n] = dv
            off += int(chunks[g]) * 128
        gsrc = (gsrc // NPC) * NPAD + (gsrc % NPC)  # index into all-gathered [8*NPAD, D]
        idx16 = gsrc.astype(np.int16).reshape(C, 8, 16).transpose(2, 0, 1).reshape(16, C * 8)
        idxs_l.append(np.ascontiguousarray(idx16))
        colv_l.append(np.ascontiguousarray(gcol.reshape(C, 128).T))
        disv_l.append(np.ascontiguousarray(gdv.reshape(C, 128).T))
    return chunks, idxs_l, colv_l, disv_l


def _build_gnn(chunks):
    import concourse.bass as bass
    from concourse import mybir, tile
    from concourse.library_config import mlp as _mlp_lib

    C = _cdiv(int(sum(chunks)), WCH) * WCH
    f32 = mybir.dt.float32
    f32r = mybir.dt.float32r
    AF = mybir.ActivationFunctionType
    ALU = mybir.AluOpType

    nc = bass.Bass("TRN2", target_bir_lowering=False, debug=False, num_devices=NCORES)
    x2sh_d = nc.dram_tensor("x2sh", [NPAD, D], f32, kind="ExternalInput").ap()
    invm_d = nc.dram_tensor("invm", [1, NPAD], f32, kind="ExternalInput").ap()
    idxs_d = nc.dram_tensor("idxs", [16, C * 8], mybir.dt.int16, kind="ExternalInput").ap()
    colv_d = nc.dram_tensor("colv", [128, C], f32, kind="ExternalInput").ap()
    disv_d = nc.dram_tensor("disv", [128, C], f32, kind="ExternalInput").ap()
    wnames = ("Wg", "Ws", "Wsn", "Wc", "Rp", "W1", "W2")
    w_d = {n: nc.dram_tensor(n, [128, 2, 2, 128], f32, kind="ExternalInput").ap() for n in wnames}
    b1_d = nc.dram_tensor("b1", [128, 2], f32, kind="ExternalInput").ap()
    ball_d = nc.dram_tensor("ball", [128, 2], f32, kind="ExternalInput").ap()
    iotaf_d = nc.dram_tensor("iotaf", [128, 128], f32, kind="ExternalInput").ap()
    idn_d = nc.dram_tensor("idn", [128, 128], f32, kind="ExternalInput").ap()
    hT_d = nc.dram_tensor("hT", [128, 2, NPAD], f32, kind="ExternalOutput").ap()

    with tile.TileContext(nc) as tc:
        with tc.tile_pool(name="persist", bufs=1) as pp, \
             tc.tile_pool(name="dramp", bufs=1, space="DRAM") as dp:
            x2T_sb = pp.tile([128, 2, NPAD], f32, tag="x2T", name="x2T_sb")
            PT = pp.tile([128, 2, NPAD], f32, tag="PT", name="PT")
            QT = pp.tile([128, 2, NPAD], f32, tag="QT", name="QT")
            invm_sb = pp.tile([128, NPAD], f32, tag="invm", name="invm_sb")
            inv1_sb = pp.tile([1, NPAD], f32, tag="inv1", name="inv1_sb")
            ones_sb = pp.tile([1, 128], f32, tag="ones", name="ones_sb")
            idxs_sb = pp.tile([128, C * 8], mybir.dt.int16, tag="idxs", name="idxs_sb")
            colv_sb = pp.tile([128, C], f32, tag="colv", name="colv_sb")
            disv_sb = pp.tile([128, C], f32, tag="disv", name="disv_sb")
            w_sb = {n: pp.tile([128, 2, 2, 128], f32, tag=n, name=n + "_sb") for n in wnames}
            b1_sb = pp.tile([128, 2], f32, tag="b1", name="b1_sb")
            ball_sb = pp.tile([128, 2], f32, tag="ball", name="ball_sb")
            iota_f = pp.tile([128, 128], f32, tag="iota_f", name="iota_f")
            idn_sb = pp.tile([128, 128], f32, tag="idn", name="idn_sb")

            x2b = dp.tile([NPAD, D], f32, tag="x2b", name="x2b")
            x2as = dp.tile([NCORES * NPAD, D], f32, tag="x2as", name="x2as")
            nc.sync.dma_start(out=x2b[:, :], in_=x2sh_d[:, :])
            nc.gpsimd.collective_compute(
                "AllGather", ALU.bypass,
                replica_groups=[list(range(NCORES))],
                ins=[x2b[:, :]], outs=[x2as[:, :]])
            nc.sync.dma_start(out=idn_sb[:, :].bitcast(f32r),
                              in_=idn_d[:, :].bitcast(f32r))
            with tc.tile_pool(name="xck", bufs=3) as xckp, \
                 tc.tile_pool(name="tpp", bufs=3, space="PSUM") as tppp:
                for g in range(GROUPS):
                    xck = xckp.tile([128, D], f32, name="xck")
                    nc.sync.dma_start(out=xck[:, :].bitcast(f32r),
                                      in_=x2sh_d[g * 128:(g + 1) * 128, :].bitcast(f32r))
                    for k in range(2):
                        tp = tppp.tile([128, 128], f32, name="tp")
                        nc.tensor.transpose(tp[:, :].bitcast(f32r),
                                            xck[:, k * 128:(k + 1) * 128].bitcast(f32r),
                                            idn_sb[:, :].bitcast(f32r))
                        nc.scalar.copy(x2T_sb[:, k, g * 128:(g + 1) * 128].bitcast(f32r),
                                       tp[:, :])
            nc.sync.dma_start(out=inv1_sb[:, :], in_=invm_d[:, :])
            nc.scalar.activation(ones_sb[:, :], idn_sb[0:1, :], AF.Copy,
                                 bias=1.0, scale=0.0)
            with tc.tile_pool(name="ivp", bufs=3, space="PSUM") as ivpp:
                for s in range(NSLICE):
                    isl = slice(s * SL, (s + 1) * SL)
                    ivps = ivpp.tile([128, SL], f32, name="ivps")
                    nc.tensor.matmul(out=ivps[:, :], lhsT=ones_sb[:, :],
                                     rhs=inv1_sb[:, isl])
                    nc.vector.tensor_copy(out=invm_sb[:, isl], in_=ivps[:, :])
            for k in range(8):
                nc.sync.dma_start(out=idxs_sb[16 * k:16 * (k + 1), :], in_=idxs_d[:, :])
            nc.sync.dma_start(out=colv_sb[:, :], in_=colv_d[:, :])
            nc.sync.dma_start(out=disv_sb[:, :], in_=disv_d[:, :])
            for n in wnames:
                nc.sync.dma_start(out=w_sb[n][:, :, :, :].bitcast(f32r),
                                  in_=w_d[n][:, :, :, :].bitcast(f32r))
            nc.sync.dma_start(out=b1_sb[:, :], in_=b1_d[:, :])
            nc.sync.dma_start(out=ball_sb[:, :], in_=ball_d[:, :])
            nc.sync.dma_start(out=iota_f[:, :], in_=iotaf_d[:, :])
            nc.gpsimd.load_library(_mlp_lib)
            nidx_reg = nc.gpsimd.to_reg(WCH * 128)

            # ---- aggregation: P' (sum + self) and Q' (sym-norm sum + self) via one-hot matmul
            with tc.tile_pool(name="gat", bufs=2) as gat, \
                 tc.tile_pool(name="ohp", bufs=4) as ohp, \
                 tc.tile_pool(name="aps", bufs=4, space="PSUM") as aps:
                gt = None
                ci = 0
                for g in range(GROUPS):
                    nch = int(chunks[g])
                    ps0 = aps.tile([128, 256], f32, name="ps0")
                    ps1 = aps.tile([128, 256], f32, name="ps1")
                    for j in range(nch):
                        if ci % WCH == 0:
                            gt = gat.tile([128, WCH, 256], f32, name="gt")
                            nc.gpsimd.dma_gather(gt[:, :, :].bitcast(f32r),
                                                 x2as[:, :].bitcast(f32r),
                                                 idxs_sb[:, ci * 8:(ci + WCH) * 8],
                                                 WCH * 128, nidx_reg, 256)
                        slot = ci % WCH
                        oh = ohp.tile([128, 256], f32, name="oh")
                        nc.vector.tensor_tensor(out=oh[:, 0:128].bitcast(f32r),
                                                in0=colv_sb[:, ci:ci + 1].to_broadcast([128, 128])[:],
                                                in1=iota_f[:, :], op=ALU.is_equal)
                        nc.scalar.activation(oh[:, 128:256].bitcast(f32r), oh[:, 0:128], AF.Copy,
                                             bias=0.0, scale=disv_sb[:, ci:ci + 1])
                        nc.tensor.matmul(out=ps0[:, :],
                                         lhsT=gt[:, slot, 0:128].bitcast(f32r),
                                         rhs=oh[:, :].bitcast(f32r),
                                         start=(j == 0), stop=(j == nch - 1),
                                         skip_group_check=True)
                        nc.tensor.matmul(out=ps1[:, :],
                                         lhsT=gt[:, slot, 128:256].bitcast(f32r),
                                         rhs=oh[:, :].bitcast(f32r),
                                         start=(j == 0), stop=(j == nch - 1),
                                         skip_group_check=True)
                        ci += 1
                    gsl = slice(g * 128, (g + 1) * 128)
                    nc.vector.tensor_copy(out=PT[:, 0, gsl].bitcast(f32r), in_=ps0[:, 0:128])
                    nc.scalar.copy(QT[:, 0, gsl].bitcast(f32r), ps0[:, 128:256])
                    nc.vector.tensor_copy(out=PT[:, 1, gsl].bitcast(f32r), in_=ps1[:, 0:128])
                    nc.scalar.copy(QT[:, 1, gsl].bitcast(f32r), ps1[:, 128:256])

            # ---- dense phase: h = Q'Wg + P'Wc + x2 Rp + relu(P'W1+b1)W2 + invm*((P'-x2)Ws) + ball
            with tc.tile_pool(name="tsbp", bufs=2) as tsbp, \
                 tc.tile_pool(name="evac", bufs=4) as evacp, \
                 tc.tile_pool(name="tps", bufs=2, space="PSUM") as tpsp, \
                 tc.tile_pool(name="yps", bufs=2, space="PSUM") as ypsp, \
                 tc.tile_pool(name="hps", bufs=2, space="PSUM") as hpsp:
                for s in range(NSLICE):
                    nsl = slice(s * SL, (s + 1) * SL)
                    tsb = tsbp.tile([128, 2, SL], f32, name="tsb")
                    for m in range(2):
                        tps = tpsp.tile([128, SL], f32, name="tps")
                        for k in range(2):
                            nc.tensor.matmul(out=tps[:, :],
                                             lhsT=w_sb["W1"][:, k, m, :].bitcast(f32r),
                                             rhs=PT[:, k, nsl].bitcast(f32r),
                                             start=(k == 0), stop=(k == 1))
                        nc.scalar.activation(tsb[:, m, :].bitcast(f32r), tps[:, :], AF.Relu,
                                             bias=b1_sb[:, m:m + 1], scale=1.0)
                    for m in range(2):
                        yps = ypsp.tile([128, SL], f32, name="yps")
                        q = 0
                        for k in range(2):
                            for wn, rhs_ap in (("Ws", PT[:, k, nsl]), ("Wsn", x2T_sb[:, k, nsl])):
                                nc.tensor.matmul(out=yps[:, :],
                                                 lhsT=w_sb[wn][:, k, m, :].bitcast(f32r),
                                                 rhs=rhs_ap.bitcast(f32r),
                                                 start=(q == 0), stop=(q == 3))
                                q += 1
                        hps = hpsp.tile([128, SL], f32, name="hps")
                        q = 0
                        for k in range(2):
                            for wn, rhs_ap in (("Wg", QT[:, k, nsl]), ("Wc", PT[:, k, nsl]),
                                               ("Rp", x2T_sb[:, k, nsl]), ("W2", tsb[:, k, :])):
                                nc.tensor.matmul(out=hps[:, :],
                                                 lhsT=w_sb[wn][:, k, m, :].bitcast(f32r),
                                                 rhs=rhs_ap.bitcast(f32r),
                                                 start=(q == 0), stop=(q == 7))
                                q += 1
                        ysb = evacp.tile([128, SL], f32, name="ysb")
                        nc.vector.tensor_tensor(out=ysb[:, :], in0=yps[:, :],
                                                in1=invm_sb[:, nsl], op=ALU.mult)
                        h1sb = evacp.tile([128, SL], f32, name="h1sb")
                        nc.scalar.activation(h1sb[:, :], hps[:, :], AF.Identity,
                                             bias=ball_sb[:, m:m + 1], scale=1.0)
                        hsb = evacp.tile([128, SL], f32, name="hsb")
                        nc.vector.tensor_tensor(out=hsb[:, :], in0=h1sb[:, :],
                                                in1=ysb[:, :], op=ALU.add)
                        nc.sync.dma_start(out=hT_d[:, m, nsl], in_=hsb[:, :])
    import bass_rust as _bass_rust
    _bass_rust.generate_event_semaphores(nc)
    from concourse.library_overlay import lower_extended_insts
    lower_extended_insts(nc)
    return nc


def _get_prog(chunks):
    key = tuple(int(c) for c in chunks)
    if key not in _PROG_CACHE:
        _PROG_CACHE[key] = _build_gnn(key)
    return _PROG_CACHE[key]


def _pack_w(W):
    return np.ascontiguousarray(
        np.asarray(W, np.float32).reshape(2, 128, 2, 128).transpose(1, 0, 2, 3))


def _layer_weights(inp, i):
    import jax
    nw = np.asarray(jax.nn.softmax(inp["na_w"][i]))
    Ws = nw[1] * np.asarray(inp["sage_Wl"][i], np.float32)
    wd = {
        "Wg": nw[0] * np.asarray(inp["gcn_W"][i], np.float32),
        "Ws": Ws,
        "Wsn": -Ws,
        "Wc": nw[3] * np.asarray(inp["gc_Wl"][i], np.float32),
        "Rp": (nw[1] * np.asarray(inp["sage_Wr"][i], np.float32)
               + nw[3] * np.asarray(inp["gc_Wr"][i], np.float32)
               - nw[3] * np.asarray(inp["gc_Wl"][i], np.float32)),
        "W1": np.asarray(inp["gin_W1"][i], np.float32),
        "W2": nw[2] * np.asarray(inp["gin_W2"][i], np.float32),
    }
    b1 = np.asarray(inp["gin_b1"][i], np.float32)
    ball = (nw[0] * np.asarray(inp["gcn_b"][i], np.float32)
            + nw[1] * np.asarray(inp["sage_b"][i], np.float32)
            + nw[2] * np.asarray(inp["gin_b2"][i], np.float32)
            + nw[3] * np.asarray(inp["gc_b"][i], np.float32))
    return wd, b1, ball


def _make_in_maps(x2, wd, b1, ball, idxs_l, colv_l, disv_l, invm_l):
    wpk = {n: _pack_w(wd[n]) for n in wd}
    b1p = np.ascontiguousarray(b1.reshape(2, 128).T)
    ballp = np.ascontiguousarray(ball.reshape(2, 128).T)
    x2c = np.ascontiguousarray(np.asarray(x2, np.float32))
    in_maps = []
    for c in range(NCORES):
        xp = np.zeros((NPAD, D), np.float32)
        xp[:NPC] = x2c[c * NPC:(c + 1) * NPC]
        im = {"x2sh": xp, "invm": invm_l[c],
              "idxs": idxs_l[c], "colv": colv_l[c], "disv": disv_l[c],
              "b1": b1p, "ball": ballp,
              "iotaf": np.ascontiguousarray(
                  np.broadcast_to(np.arange(128, dtype=np.float32), (128, 128))),
              "idn": np.eye(128, dtype=np.float32)}
        im.update(wpk)
        in_maps.append(im)
    return in_maps


def _run_layer(nc, in_maps):
    import time
    from concourse.bass_utils import run_bass_kernel_spmd
    t0 = time.monotonic_ns()
    res = run_bass_kernel_spmd(nc, in_maps, list(range(NCORES)), trace=TRACE)
    LAST_WALL_NS.append(time.monotonic_ns() - t0)
    ns = getattr(res, "exec_time_ns", None)
    if ns:
        LAST_EXEC_NS.append(ns)
    h = np.empty((N, D), np.float32)
    for c in range(NCORES):
        hT = np.asarray(res.results[c]["hT"])
        h[c * NPC:(c + 1) * NPC] = hT.transpose(2, 1, 0).reshape(NPAD, D)[:NPC]
    return h


def kernel(**inputs):
    import jax
    import jax.numpy as jnp
    inp = inputs
    cpu = jax.devices("cpu")[0]

    src = np.asarray(inp["edge_index"][0]).astype(np.int64)
    dst = np.asarray(inp["edge_index"][1]).astype(np.int64)
    deg = np.zeros(N, np.float32)
    np.add.at(deg, dst, np.float32(1.0))
    invmax = (1.0 / np.maximum(deg, 1.0)).astype(np.float32)

    with jax.default_device(cpu):
        dis = np.asarray(jax.lax.rsqrt(jnp.asarray(deg) + 1.0))

        # layer 0: all rows of h0 are identical -> single-row compute
        emb0 = np.asarray(inp["emb"])[0]
        h0b = jnp.asarray(np.broadcast_to(emb0, (N, D)))
        sw0 = jax.nn.softmax(inp["se_w"][0, :1], axis=-1)
        fw0 = jax.nn.softmax(inp["fu_w"][0])
        st0 = sw0[0, 1] * h0b
        fused0 = fw0[0] * st0 + fw0[1] * st0 + fw0[2] * st0
        x2_0 = jax.nn.elu(_bn(fused0, inp["bn_gamma"][0], inp["bn_beta"][0]))
        u = np.asarray(x2_0)[0]

        # structured h1: rank-1 + per-degree-class closed form
        nw = jax.nn.softmax(inp["na_w"][0])
        uj = jnp.asarray(u)
        w_gcn = uj @ inp["gcn_W"][0]
        w_sl = uj @ inp["sage_Wl"][0]
        w_sr = uj @ inp["sage_Wr"][0]
        w_gl = uj @ inp["gc_Wl"][0]
        w_gr = uj @ inp["gc_Wr"][0]
        t = np.zeros(N, np.float32)
        np.add.at(t, dst, dis[src].astype(np.float32))
        degs_unique = np.unique(deg)
        uW1 = uj @ inp["gin_W1"][0]
        gin_rows = {}
        for dv in degs_unique:
            hh = (1.0 + np.float32(dv)) * uW1
            gin_rows[float(dv)] = np.asarray(
                jax.nn.relu(hh + inp["gin_b1"][0]) @ inp["gin_W2"][0] + inp["gin_b2"][0])
        gin_tab = np.stack([gin_rows[float(dv)] for dv in degs_unique])
        deg_idx = np.searchsorted(degs_unique, deg)
        gcn_scal = (dis * t + dis * dis).astype(np.float32)
        sage_scal = (deg / np.maximum(deg, 1.0)).astype(np.float32)
        h1 = (nw[0] * (jnp.asarray(gcn_scal)[:, None] * w_gcn[None, :])
              + nw[1] * (jnp.asarray(sage_scal)[:, None] * w_sl[None, :] + w_sr[None, :])
              + nw[2] * jnp.asarray(gin_tab)[jnp.asarray(deg_idx)]
              + nw[3] * (jnp.asarray(deg)[:, None] * w_gl[None, :] + w_gr[None, :]))
        h1 = jnp.asarray(np.asarray(h1, np.float32))

    chunks, idxs_l, colv_l, disv_l = _edge_prep(src, dst, dis)
    nc = _get_prog(chunks)
    invm_l = []
    for c in range(NCORES):
        ivp = np.zeros(NPAD, np.float32)
        ivp[:NPC] = invmax[c * NPC:(c + 1) * NPC]
        invm_l.append(ivp[None, :])

    h_list = [h0b, h1]
    for i in (1, 2):
        with jax.default_device(cpu):
            sw = jax.nn.softmax(inp["se_w"][i, :i + 1], axis=-1)
            st = jnp.stack([sw[j, 1] * h_list[j] for j in range(i + 1)])
            fw = jax.nn.softmax(inp["fu_w"][i])
            fused = fw[0] * st.sum(0) + fw[1] * st.mean(0) + fw[2] * st.max(0)
            x2 = np.asarray(jax.nn.elu(_bn(fused, inp["bn_gamma"][i], inp["bn_beta"][i])),
                            np.float32)
            wd, b1, ball = _layer_weights(inp, i)
        in_maps = _make_in_maps(x2, wd, b1, ball, idxs_l, colv_l, disv_l, invm_l)
        h = _run_layer(nc, in_maps)
        h_list.append(jnp.asarray(h))

    with jax.default_device(cpu):
        i = 3
        sw = jax.nn.softmax(inp["se_w"][i, :i + 1], axis=-1)
        st = jnp.stack([sw[j, 1] * h_list[j] for j in range(i + 1)])
        fw = jax.nn.softmax(inp["fu_w"][i])
        fused = fw[0] * st.sum(0) + fw[1] * st.mean(0) + fw[2] * st.max(0)
        x2 = jax.nn.elu(_bn(fused, inp["bn_gamma"][i], inp["bn_beta"][i]))
        pooled = jax.ops.segment_sum(x2, jnp.asarray(inp["batch"]), num_segments=NG)
        out = np.asarray(pooled @ inp["cls_W"] + inp["cls_b"], np.float32)
    return out
